# revision 9
# baseline (speedup 1.0000x reference)
"""BatchHardTripletLoss on 8 Trainium2 NeuronCores.

Strategy (batch/row sharding): core c owns anchor rows [512c, 512c+512).
All tensors are rolled by 512c rows on the host so local row i == global
row 512c+i and the self-match diagonal is at a static column block.

Host-side input marshalling (layout/encoding prep only):
  - per tensor Y: Y8 = fp8(Y) and Q8 = fp8 rows of (Y^2 - 1) quantized
    with error feedback along d so sum_d Q8[j,d] == ||y_j||^2 - 128 to
    ~fp32 accuracy.
  - ypk{y} [128, 2, 4096]: k-tile 0 = Y8^T, k-tile 1 = Q8^T.
  - lhspack [128, 4, 2, 128]: per m-block, k-tile 0 = own-anchor fp8
    columns, k-tile 1 = const -0.5.
  - fp32 transposed slices of the core's own anchors/positives for the
    exact row stats (a2, distance_pos).

Device (per core):
  - One fp8 DoubleRow (K=256) matmul per 512-col bank computes
    psum = a_i.y_j - 0.5(||y_j||^2 - 128) at 0.5 cyc/col; an extra
    DoubleRow matmul adds -448 on the self-diagonal (mask).
    hardest-neg: d2_min = a2 + 128 - 2*max_j psum.
  - psum drains split across engines: VectorE tensor_tensor_reduce
    (max-of-halves + accumulated max) and GpSimd tensor_tensor max
    into bf16 partials that VectorE finishes in 2x mode.
  - Row stats a2/dpsq via per-block column-sum matmuls (fp32, exact).
  - Tail: Sqrt on ScalarE (one act table), softplus(x) computed as
    max(x,0) + Pade33(ln(1+e^-|x|)) so only one more table (Exp) is
    ever loaded.  Each core emits the sum of its 512 row losses; the
    host sums 8 partials and divides by 4096.
"""

import os
import sys

if "/opt/trn_rl_repo" not in sys.path:
    sys.path.insert(0, "/opt/trn_rl_repo")

from contextlib import ExitStack

import numpy as np
import ml_dtypes

import concourse.bass as bass
import concourse.tile as tile
from concourse import bacc, bass_utils, mybir

F32 = mybir.dt.float32
F8 = mybir.dt.float8e4
BF16 = mybir.dt.bfloat16
AF = mybir.ActivationFunctionType
ALU = mybir.AluOpType
DR = mybir.MatmulPerfMode.DoubleRow
# e4m3fn shares encodings with e4m3 for |v| <= 240 (all values used here);
# XLA/PJRT accepts the fn variant.
NPF8 = ml_dtypes.float8_e4m3fn

B, D, NCORES = 4096, 128, 8
RB = B // NCORES        # 512 rows per core
MT = RB // 128          # 4 m-blocks per core
EPS = 1e-12
NEG = -3.0e38

_CACHE: dict = {}


def _build():
    nc = bacc.Bacc("TRN2", target_bir_lowering=False, debug=False)

    lhs_d = nc.dram_tensor("lhspack", [128, MT, 2, 128], F8,
                           kind="ExternalInput").ap()
    eye_d = nc.dram_tensor("eyepack", [128, 2, 128], F8,
                           kind="ExternalInput").ap()
    ibf_d = nc.dram_tensor("ibufpack", [128, 2, 1024], F8,
                           kind="ExternalInput").ap()
    asl_d = nc.dram_tensor("aslice", [128, RB], F32, kind="ExternalInput").ap()
    psl_d = nc.dram_tensor("pslice", [128, RB], F32, kind="ExternalInput").ap()
    ypk_d = [nc.dram_tensor(f"ypk{y}", [128, 2, B], F8,
                            kind="ExternalInput").ap() for y in range(3)]
    out_d = nc.dram_tensor("out", [1, 1], F32, kind="ExternalOutput").ap()

    with tile.TileContext(nc) as tc:
        with ExitStack() as ctx:
            _emit(ctx, tc, nc, lhs_d, eye_d, ibf_d, asl_d, psl_d, ypk_d, out_d)
    nc.compile()
    return nc


def _emit(ctx, tc, nc, lhs_d, eye_d, ibf_d, asl_d, psl_d, ypk_d, out_d):
    const = ctx.enter_context(tc.tile_pool(name="const", bufs=1))
    inp = ctx.enter_context(tc.tile_pool(name="inp", bufs=1))
    gtp = ctx.enter_context(tc.tile_pool(name="gtp", bufs=3))
    fin = ctx.enter_context(tc.tile_pool(name="fin", bufs=1))
    scr = ctx.enter_context(tc.tile_pool(name="scr", bufs=2))
    mpsum = ctx.enter_context(tc.tile_pool(name="mpsum", bufs=2, space="PSUM"))

    lhsp = inp.tile([128, MT, 2, 128], F8, tag="lhsp")
    eyep = inp.tile([128, 2, 128], F8, tag="eyep")
    ibufp = inp.tile([128, 2, 1024], F8, tag="ibufp")
    asl = inp.tile([128, RB], F32, tag="asl")
    psl = inp.tile([128, RB], F32, tag="psl")
    ypk = [inp.tile([128, 2, B], F8, tag=f"ypk{y}", name=f"ypk{y}")
           for y in range(3)]

    ones_col = const.tile([128, 1], F32, tag="ones_col")
    nc.vector.memset(ones_col[:], 1.0)

    # ---- input DMAs: small/stats tensors first, then ypk in use order ----
    nc.sync.dma_start(lhsp[:], lhs_d)
    nc.sync.dma_start(eyep[:], eye_d)
    nc.sync.dma_start(ibufp[:], ibf_d)
    nc.sync.dma_start(asl[:], asl_d)
    nc.sync.dma_start(psl[:], psl_d)
    CH = 1024
    for y in range(3):
        for g in range(B // CH):
            nc.sync.dma_start(ypk[y][:, :, CH * g:CH * (g + 1)],
                              ypk_d[y][:, :, CH * g:CH * (g + 1)])

    # ---- row stats: a2 / dpsq by per-block column-sum matmuls ----
    asq = scr.tile([128, RB], F32, tag="asq")
    nc.scalar.activation(asq[:], asl[:], AF.Square)
    dif = scr.tile([128, RB], F32, tag="dif")
    nc.vector.tensor_tensor(out=dif[:], in0=asl[:], in1=psl[:],
                            op=ALU.subtract)
    difsq = scr.tile([128, RB], F32, tag="difsq")
    nc.scalar.activation(difsq[:], dif[:], AF.Square)
    sp = mpsum.tile([128, 2048], F32, tag="grp", name="spstats")
    for m in range(MT):
        nc.tensor.matmul(sp[:, m:m + 1], asq[:, 128 * m:128 * (m + 1)],
                         ones_col[:], start=(m == 0), stop=False)
    for m in range(MT):
        nc.tensor.matmul(sp[:, MT + m:MT + m + 1],
                         difsq[:, 128 * m:128 * (m + 1)], ones_col[:],
                         start=False, stop=(m == MT - 1))
    scol = fin.tile([128, 2 * MT], F32, tag="scol")
    nc.scalar.activation(scol[:], sp[:, 0:2 * MT], AF.Copy)
    nc.vector.tensor_scalar_max(out=scol[:, MT:2 * MT],
                                in0=scol[:, MT:2 * MT], scalar1=EPS)

    # ---- working tiles for the reduction/tail ----
    vcol = fin.tile([128, 8 * MT], F32, tag="vcol")
    nc.vector.memset(vcol[:], NEG)
    maxv = fin.tile([128, MT], F32, tag="maxv")
    hnsq = fin.tile([128, MT], F32, tag="hnsq")
    hn = fin.tile([128, MT], F32, tag="hn")
    dp = fin.tile([128, MT], F32, tag="dp")
    xcol = fin.tile([128, MT], F32, tag="xcol")

    sqrt_instrs = []
    slot = {m: 0 for m in range(MT)}

    def vslot(m):
        s = slot[m]
        slot[m] += 1
        assert s < 8
        return vcol[:, 8 * m + s:8 * m + s + 1]

    # Drain routes per unit (PSUM is readable only by VectorE and ScalarE,
    # one PSUM operand per instruction, always at 1 elem/cycle/lane):
    #   V — vector.tensor_reduce direct from psum (~2.26us/unit @0.96GHz)
    #   S — scalar ACT copies psum -> bf16 SBUF (~1.85us/unit @1.2GHz);
    #       vector finishes each bf16 tile with one tensor_scalar max-accum
    #       (single-src bf16 SBUF op -> 4x mode, ~0.53us/unit).
    # (tensor_tensor_reduce with a min/max accum op hard-crashes TRN2 —
    # probed; tensor_scalar's op1 accumulation is the working path.)
    def route(y, m, h):
        if h == 1:
            return "S"
        return "V" if (y < 2 or m == 0) else "S"

    def finish_st(m, st, name):
        tout = scr.tile([128, 2048], BF16, tag="tout", name=f"to{name}")
        nc.vector.tensor_scalar(out=tout[:], in0=st[:], scalar1=NEG,
                                scalar2=None, op0=ALU.max, op1=ALU.max,
                                accum_out=vslot(m))

    # ---- main loop: 24 psum units of [128, 2048] over (y, m, half) ----
    for y in range(3):
        for m in range(MT):
            for h in range(2):
                g = mpsum.tile([128, 2048], F32, tag="grp", name=f"g{y}{m}{h}")
                c0 = 2048 * h
                if h == 0 and y < 2:
                    nc.tensor.matmul(g[:, 0:512], eyep[:],
                                     ibufp[:, :, 512 - 128 * m:1024 - 128 * m],
                                     start=True, stop=False, perf_mode=DR)
                for k in range(4):
                    masked0 = (k == 0 and h == 0 and y < 2)
                    nc.tensor.matmul(g[:, 512 * k:512 * (k + 1)], lhsp[:, m],
                                     ypk[y][:, :, c0 + 512 * k:c0 + 512 * (k + 1)],
                                     start=not masked0, stop=True, perf_mode=DR)
                if route(y, m, h) == "V":
                    nc.vector.tensor_reduce(out=vslot(m), in_=g[:],
                                            axis=mybir.AxisListType.X,
                                            op=ALU.max)
                else:
                    st = gtp.tile([128, 2048], BF16, tag="st",
                                  name=f"st{y}{m}{h}")
                    nc.scalar.activation(st[:], g[:], AF.Copy)
                    finish_st(m, st, f"{y}{m}{h}")

            if y == 2:
                # per-m tail: hardest-neg^2, then Sqrt (table loaded once)
                nc.vector.tensor_reduce(out=maxv[:, m:m + 1],
                                        in_=vcol[:, 8 * m:8 * m + 8],
                                        axis=mybir.AxisListType.X, op=ALU.max)
                nc.vector.tensor_scalar(out=hnsq[:, m:m + 1],
                                        in0=maxv[:, m:m + 1], scalar1=-2.0,
                                        scalar2=128.0, op0=ALU.mult,
                                        op1=ALU.add)
                nc.vector.tensor_tensor(out=hnsq[:, m:m + 1],
                                        in0=hnsq[:, m:m + 1],
                                        in1=scol[:, m:m + 1], op=ALU.add)
                nc.vector.tensor_scalar_max(out=hnsq[:, m:m + 1],
                                            in0=hnsq[:, m:m + 1], scalar1=EPS)
                sqrt_instrs.append(
                    nc.scalar.activation(hn[:, m:m + 1], hnsq[:, m:m + 1],
                                         AF.Sqrt))
                sqrt_instrs.append(
                    nc.scalar.activation(dp[:, m:m + 1],
                                         scol[:, MT + m:MT + m + 1], AF.Sqrt))
                nc.vector.tensor_tensor(out=xcol[:, m:m + 1],
                                        in0=dp[:, m:m + 1],
                                        in1=hn[:, m:m + 1], op=ALU.subtract)

    # ---- softplus tail: loss = max(x,0) + Pade33(ln(1+u)), u = e^-|x| ----
    zneg = fin.tile([128, MT], F32, tag="zneg")
    nc.vector.tensor_scalar(out=zneg[:], in0=xcol[:], scalar1=-1.0,
                            scalar2=None, op0=ALU.mult)
    nc.vector.tensor_tensor(out=zneg[:], in0=xcol[:], in1=zneg[:], op=ALU.min)
    u = fin.tile([128, MT], F32, tag="u")
    i_exp = nc.scalar.activation(u[:], zneg[:], AF.Exp)
    # keep the Exp after every Sqrt so the act table only switches once
    from concourse.bass import _add_dep_helper
    for si in sqrt_instrs:
        _add_dep_helper(i_exp.ins, si.ins, sync=False, reason="act table order")

    u2 = fin.tile([128, MT], F32, tag="u2")
    nc.vector.tensor_tensor(out=u2[:], in0=u[:], in1=u[:], op=ALU.mult)
    t1 = fin.tile([128, MT], F32, tag="t1")
    nc.vector.tensor_scalar(out=t1[:], in0=u[:], scalar1=60.0, scalar2=60.0,
                            op0=ALU.mult, op1=ALU.add)
    t2 = fin.tile([128, MT], F32, tag="t2")
    nc.vector.tensor_scalar(out=t2[:], in0=u2[:], scalar1=11.0, scalar2=None,
                            op0=ALU.mult)
    nc.vector.tensor_tensor(out=t1[:], in0=t1[:], in1=t2[:], op=ALU.add)
    num = fin.tile([128, MT], F32, tag="num")
    nc.vector.tensor_tensor(out=num[:], in0=t1[:], in1=u[:], op=ALU.mult)
    den = fin.tile([128, MT], F32, tag="den")
    nc.vector.tensor_scalar(out=den[:], in0=u[:], scalar1=90.0, scalar2=60.0,
                            op0=ALU.mult, op1=ALU.add)
    nc.vector.tensor_scalar(out=t2[:], in0=u[:], scalar1=3.0, scalar2=36.0,
                            op0=ALU.mult, op1=ALU.add)
    nc.vector.tensor_tensor(out=t2[:], in0=t2[:], in1=u2[:], op=ALU.mult)
    nc.vector.tensor_tensor(out=den[:], in0=den[:], in1=t2[:], op=ALU.add)
    rden = fin.tile([128, MT], F32, tag="rden")
    nc.vector.reciprocal(rden[:], den[:])
    lg = fin.tile([128, MT], F32, tag="lg")
    nc.vector.tensor_tensor(out=lg[:], in0=num[:], in1=rden[:], op=ALU.mult)
    relu = fin.tile([128, MT], F32, tag="relu")
    nc.vector.tensor_scalar_max(out=relu[:], in0=xcol[:], scalar1=0.0)
    nc.vector.tensor_tensor(out=lg[:], in0=lg[:], in1=relu[:], op=ALU.add)
    lsum = fin.tile([128, 1], F32, tag="lsum")
    nc.vector.tensor_reduce(out=lsum[:], in_=lg[:],
                            axis=mybir.AxisListType.X, op=ALU.add)
    ps = mpsum.tile([128, 2048], F32, tag="grp", name="psfinal")
    nc.tensor.matmul(ps[0:1, 0:1], lsum[:], ones_col[:], start=True, stop=True)
    res = fin.tile([1, 1], F32, tag="res")
    nc.scalar.activation(res[:], ps[0:1, 0:1], AF.Copy)
    nc.sync.dma_start(out_d, res[:])


def _get_nc():
    if "nc" not in _CACHE:
        _CACHE["nc"] = _build()
    return _CACHE["nc"]


def _feedback_quant(x):
    """fp8-quantize rows of x with error feedback along the last axis so
    each row's fp8 sum tracks the fp32 row sum."""
    out = np.empty(x.shape, dtype=NPF8)
    carry = np.zeros(x.shape[0], dtype=np.float32)
    for d in range(x.shape[1]):
        v = x[:, d] + carry
        q = v.astype(NPF8)
        out[:, d] = q
        carry = v - q.astype(np.float32)
    return out


def _host_pack(A, P, N):
    Ys = [A, P, N]
    Y8 = [Y.astype(NPF8) for Y in Ys]
    Q8 = [_feedback_quant(Y * Y - 1.0) for Y in Ys]

    eye = (np.eye(128, dtype=np.float32) * -224.0).astype(NPF8)
    eyepack = np.ascontiguousarray(np.stack([eye, eye], axis=1))
    ib = np.zeros((128, 1024), dtype=np.float32)
    ib[:, 512:640] = np.eye(128, dtype=np.float32)
    ib8 = ib.astype(NPF8)
    ibufpack = np.ascontiguousarray(np.stack([ib8, ib8], axis=1))

    in_maps = []
    for c in range(NCORES):
        r = RB * c
        idx = np.r_[r:B, 0:r]
        m = {"eyepack": eyepack, "ibufpack": ibufpack}
        for y in range(3):
            v = Y8[y][idx].T
            q = Q8[y][idx].T
            m[f"ypk{y}"] = np.ascontiguousarray(np.stack([v, q], axis=1))
        ownT = Y8[0][idx][:RB].T          # [128, 512] fp8
        lhspack = np.empty((128, MT, 2, 128), dtype=NPF8)
        for mm in range(MT):
            lhspack[:, mm, 0, :] = ownT[:, 128 * mm:128 * (mm + 1)]
        lhspack[:, :, 1, :] = np.float32(-0.5)
        m["lhspack"] = lhspack
        m["aslice"] = np.ascontiguousarray(A[idx][:RB].T)
        m["pslice"] = np.ascontiguousarray(P[idx][:RB].T)
        in_maps.append(m)
    return in_maps


def kernel(rep_anchor, rep_pos, rep_neg):
    A = np.ascontiguousarray(rep_anchor, dtype=np.float32)
    P = np.ascontiguousarray(rep_pos, dtype=np.float32)
    N = np.ascontiguousarray(rep_neg, dtype=np.float32)

    nc = _get_nc()
    in_maps = _host_pack(A, P, N)
    res = bass_utils.run_bass_kernel_spmd(nc, in_maps,
                                          core_ids=list(range(NCORES)))
    total = np.float64(0.0)
    for c in range(NCORES):
        total += np.float64(res.results[c]["out"][0, 0])
    return np.float32(total / B)


# revision 13
# speedup vs baseline: 1.0874x; 1.0874x over previous
"""BatchHardTripletLoss on 8 Trainium2 NeuronCores.

Strategy (batch/row sharding): core c owns anchor rows [512c, 512c+512).
All tensors are rolled by 512c rows on the host so local row i == global
row 512c+i and the self-match diagonal is at a static column block.

Host-side input marshalling (layout/encoding prep only):
  - per tensor Y: Y8 = fp8(Y) and Q8 = fp8 rows of (Y^2 - 1) quantized
    with error feedback along d so sum_d Q8[j,d] == ||y_j||^2 - 128 to
    ~fp32 accuracy.
  - ypk{y} [128, 2, 4096]: k-tile 0 = Y8^T, k-tile 1 = Q8^T.
  - lhspack [128, 4, 2, 128]: per m-block, k-tile 0 = own-anchor fp8
    columns, k-tile 1 = const -0.5.
  - fp32 transposed slices of the core's own anchors/positives for the
    exact row stats (a2, distance_pos).

Device (per core):
  - One fp8 DoubleRow (K=256) matmul per 512-col bank computes
    psum = a_i.y_j - 0.5(||y_j||^2 - 128) at 0.5 cyc/col; an extra
    DoubleRow matmul adds -448 on the self-diagonal (mask).
    hardest-neg: d2_min = a2 + 128 - 2*max_j psum.
  - psum drains split across engines: VectorE tensor_tensor_reduce
    (max-of-halves + accumulated max) and GpSimd tensor_tensor max
    into bf16 partials that VectorE finishes in 2x mode.
  - Row stats a2/dpsq via per-block column-sum matmuls (fp32, exact).
  - Tail: Sqrt on ScalarE (one act table), softplus(x) computed as
    max(x,0) + Pade33(ln(1+e^-|x|)) so only one more table (Exp) is
    ever loaded.  Each core emits the sum of its 512 row losses; the
    host sums 8 partials and divides by 4096.
"""

import os
import sys

if "/opt/trn_rl_repo" not in sys.path:
    sys.path.insert(0, "/opt/trn_rl_repo")

from contextlib import ExitStack

import numpy as np
import ml_dtypes

import concourse.bass as bass
import concourse.tile as tile
from concourse import bacc, bass_utils, mybir

F32 = mybir.dt.float32
F8 = mybir.dt.float8e4
BF16 = mybir.dt.bfloat16
AF = mybir.ActivationFunctionType
ALU = mybir.AluOpType
DR = mybir.MatmulPerfMode.DoubleRow
# e4m3fn shares encodings with e4m3 for |v| <= 240 (all values used here);
# XLA/PJRT accepts the fn variant.
NPF8 = ml_dtypes.float8_e4m3fn

B, D, NCORES = 4096, 128, 8
RB = B // NCORES        # 512 rows per core
MT = RB // 128          # 4 m-blocks per core
EPS = 1e-12
NEG = -3.0e38

_CACHE: dict = {}


def _build():
    nc = bacc.Bacc("TRN2", target_bir_lowering=False, debug=False)

    lhs_d = nc.dram_tensor("lhspack", [128, MT, 2, 128], F8,
                           kind="ExternalInput").ap()
    eye_d = nc.dram_tensor("eyepack", [128, 2, 128], F8,
                           kind="ExternalInput").ap()
    ibf_d = nc.dram_tensor("ibufpack", [128, 2, 1024], F8,
                           kind="ExternalInput").ap()
    asl_d = nc.dram_tensor("aslice", [128, RB], F32, kind="ExternalInput").ap()
    psl_d = nc.dram_tensor("pslice", [128, RB], F32, kind="ExternalInput").ap()
    ypk_d = [nc.dram_tensor(f"ypk{y}", [128, 2, B], F8,
                            kind="ExternalInput").ap() for y in range(3)]
    out_d = nc.dram_tensor("out", [1, 1], F32, kind="ExternalOutput").ap()

    with tile.TileContext(nc) as tc:
        with ExitStack() as ctx:
            _emit(ctx, tc, nc, lhs_d, eye_d, ibf_d, asl_d, psl_d, ypk_d, out_d)
    nc.compile()
    return nc


def _emit(ctx, tc, nc, lhs_d, eye_d, ibf_d, asl_d, psl_d, ypk_d, out_d):
    const = ctx.enter_context(tc.tile_pool(name="const", bufs=1))
    inp = ctx.enter_context(tc.tile_pool(name="inp", bufs=1))
    gtp = ctx.enter_context(tc.tile_pool(name="gtp", bufs=3))
    fin = ctx.enter_context(tc.tile_pool(name="fin", bufs=1))
    scr = ctx.enter_context(tc.tile_pool(name="scr", bufs=2))
    mpsum = ctx.enter_context(tc.tile_pool(name="mpsum", bufs=2, space="PSUM"))

    lhsp = inp.tile([128, MT, 2, 128], F8, tag="lhsp")
    eyep = inp.tile([128, 2, 128], F8, tag="eyep")
    ibufp = inp.tile([128, 2, 1024], F8, tag="ibufp")
    asl = inp.tile([128, RB], F32, tag="asl")
    psl = inp.tile([128, RB], F32, tag="psl")
    ypk = [inp.tile([128, 2, B], F8, tag=f"ypk{y}", name=f"ypk{y}")
           for y in range(3)]

    ones_col = const.tile([128, 1], F32, tag="ones_col")
    nc.vector.memset(ones_col[:], 1.0)

    # ---- input DMAs: tiny weights first, then ypk in use order (big
    #      contiguous transfers), stats slices last ----
    nc.sync.dma_start(lhsp[:], lhs_d)
    nc.sync.dma_start(eyep[:], eye_d)
    nc.sync.dma_start(ibufp[:], ibf_d)
    nc.sync.dma_start(asl[:], asl_d)
    nc.sync.dma_start(psl[:], psl_d)
    nc.sync.dma_start(ypk[0][:, :, 0:2048], ypk_d[0][:, :, 0:2048])
    nc.sync.dma_start(ypk[0][:, :, 2048:B], ypk_d[0][:, :, 2048:B])
    nc.sync.dma_start(ypk[1][:], ypk_d[1])
    nc.sync.dma_start(ypk[2][:], ypk_d[2])

    # ---- row stats: a2 / dpsq by per-block column-sum matmuls ----
    asq = scr.tile([128, RB], F32, tag="asq")
    nc.scalar.activation(asq[:], asl[:], AF.Square)
    dif = scr.tile([128, RB], F32, tag="dif")
    nc.vector.tensor_tensor(out=dif[:], in0=asl[:], in1=psl[:],
                            op=ALU.subtract)
    difsq = scr.tile([128, RB], F32, tag="difsq")
    nc.scalar.activation(difsq[:], dif[:], AF.Square)
    sp = mpsum.tile([128, 2048], F32, tag="grp", name="spstats")
    for m in range(MT):
        nc.tensor.matmul(sp[:, m:m + 1], asq[:, 128 * m:128 * (m + 1)],
                         ones_col[:], start=(m == 0), stop=False)
    for m in range(MT):
        nc.tensor.matmul(sp[:, MT + m:MT + m + 1],
                         difsq[:, 128 * m:128 * (m + 1)], ones_col[:],
                         start=False, stop=(m == MT - 1))
    scol = fin.tile([128, 2 * MT], F32, tag="scol")
    nc.scalar.activation(scol[:], sp[:, 0:2 * MT], AF.Copy)
    nc.vector.tensor_scalar_max(out=scol[:, MT:2 * MT],
                                in0=scol[:, MT:2 * MT], scalar1=EPS)

    # ---- working tiles for the reduction/tail ----
    vcol = fin.tile([128, 8 * MT], F32, tag="vcol")
    nc.vector.memset(vcol[:], NEG)
    maxv = fin.tile([128, MT], F32, tag="maxv")
    hnsq = fin.tile([128, MT], F32, tag="hnsq")
    hn = fin.tile([128, MT], F32, tag="hn")
    dp = fin.tile([128, MT], F32, tag="dp")
    xcol = fin.tile([128, MT], F32, tag="xcol")

    sqrt_instrs = []
    slot = {m: 0 for m in range(MT)}

    def vslot(m):
        s = slot[m]
        slot[m] += 1
        assert s < 8
        return vcol[:, 8 * m + s:8 * m + s + 1]

    # Drain routes per unit (PSUM is readable only by VectorE and ScalarE,
    # one PSUM operand per instruction, always at 1 elem/cycle/lane; any
    # DVE reduction/accumulation datapath is also 1x):
    #   V — vector.tensor_reduce direct from psum (~2.26us/unit @0.96GHz)
    #   S — scalar ACT copies psum -> bf16 SBUF (~1.9us/unit @1.2GHz);
    #       vector folds the bf16 tiles into a per-m running max with
    #       tensor_tensor max links (bf16 SBUF -> 2x mode, ~1.07us/unit),
    #       finishing each m with one fold + 1024-wide reduce.
    # (tensor_tensor_reduce with a min/max accum op hard-crashes TRN2 —
    # probed; these are the fastest working paths.)
    def route(y, m, h):
        if h == 0 and (y == 0 or (y == 1 and m == 0)):
            return "V"
        return "S"

    run = {m: None for m in range(MT)}    # running bf16 max per m

    def chain(m, st):
        nr = gtp.tile([128, 2048], BF16, tag=f"run{m}", name=f"run{m}")
        if run[m] is None:
            # chain head: 4x-mode bf16 copy, cheap
            nc.vector.tensor_scalar(out=nr[:], in0=st[:], scalar1=NEG,
                                    scalar2=None, op0=ALU.max)
        else:
            nc.vector.tensor_tensor(out=nr[:], in0=run[m][:], in1=st[:],
                                    op=ALU.max)
        run[m] = nr

    # ---- main loop: 24 psum units of [128, 2048] over (y, m, half) ----
    for y in range(3):
        for m in range(MT):
            for h in range(2):
                g = mpsum.tile([128, 2048], F32, tag="grp", name=f"g{y}{m}{h}")
                c0 = 2048 * h
                if h == 0 and y < 2:
                    nc.tensor.matmul(g[:, 0:512], eyep[:],
                                     ibufp[:, :, 512 - 128 * m:1024 - 128 * m],
                                     start=True, stop=False, perf_mode=DR)
                for k in range(4):
                    masked0 = (k == 0 and h == 0 and y < 2)
                    nc.tensor.matmul(g[:, 512 * k:512 * (k + 1)], lhsp[:, m],
                                     ypk[y][:, :, c0 + 512 * k:c0 + 512 * (k + 1)],
                                     start=not masked0, stop=True, perf_mode=DR)
                if route(y, m, h) == "V":
                    nc.vector.tensor_reduce(out=vslot(m), in_=g[:],
                                            axis=mybir.AxisListType.X,
                                            op=ALU.max)
                else:
                    st = gtp.tile([128, 2048], BF16, tag="st",
                                  name=f"st{y}{m}{h}")
                    nc.scalar.activation(st[:], g[:], AF.Copy)
                    chain(m, st)

            if y == 2:
                # close the chain: fold halves (2x) then 1024-wide reduce
                r = run[m]
                half = scr.tile([128, 1024], BF16, tag="half", name=f"hf{m}")
                nc.vector.tensor_tensor(out=half[:], in0=r[:, 0:1024],
                                        in1=r[:, 1024:2048], op=ALU.max)
                nc.vector.tensor_reduce(out=vslot(m), in_=half[:],
                                        axis=mybir.AxisListType.X, op=ALU.max)
                # per-m tail: hardest-neg^2, then Sqrt (table loaded once)
                nc.vector.tensor_reduce(out=maxv[:, m:m + 1],
                                        in_=vcol[:, 8 * m:8 * m + 8],
                                        axis=mybir.AxisListType.X, op=ALU.max)
                nc.vector.tensor_scalar(out=hnsq[:, m:m + 1],
                                        in0=maxv[:, m:m + 1], scalar1=-2.0,
                                        scalar2=128.0, op0=ALU.mult,
                                        op1=ALU.add)
                nc.vector.tensor_tensor(out=hnsq[:, m:m + 1],
                                        in0=hnsq[:, m:m + 1],
                                        in1=scol[:, m:m + 1], op=ALU.add)
                nc.vector.tensor_scalar_max(out=hnsq[:, m:m + 1],
                                            in0=hnsq[:, m:m + 1], scalar1=EPS)
                sqrt_instrs.append(
                    nc.scalar.activation(hn[:, m:m + 1], hnsq[:, m:m + 1],
                                         AF.Sqrt))
                sqrt_instrs.append(
                    nc.scalar.activation(dp[:, m:m + 1],
                                         scol[:, MT + m:MT + m + 1], AF.Sqrt))
                nc.vector.tensor_tensor(out=xcol[:, m:m + 1],
                                        in0=dp[:, m:m + 1],
                                        in1=hn[:, m:m + 1], op=ALU.subtract)

    # ---- softplus tail: loss = max(x,0) + Pade33(ln(1+u)), u = e^-|x| ----
    zneg = fin.tile([128, MT], F32, tag="zneg")
    nc.vector.tensor_scalar(out=zneg[:], in0=xcol[:], scalar1=-1.0,
                            scalar2=None, op0=ALU.mult)
    nc.vector.tensor_tensor(out=zneg[:], in0=xcol[:], in1=zneg[:], op=ALU.min)
    u = fin.tile([128, MT], F32, tag="u")
    i_exp = nc.scalar.activation(u[:], zneg[:], AF.Exp)
    # keep the Exp after every Sqrt so the act table only switches once
    from concourse.bass import _add_dep_helper
    for si in sqrt_instrs:
        _add_dep_helper(i_exp.ins, si.ins, sync=False, reason="act table order")

    u2 = fin.tile([128, MT], F32, tag="u2")
    nc.vector.tensor_tensor(out=u2[:], in0=u[:], in1=u[:], op=ALU.mult)
    t1 = fin.tile([128, MT], F32, tag="t1")
    nc.vector.tensor_scalar(out=t1[:], in0=u[:], scalar1=60.0, scalar2=60.0,
                            op0=ALU.mult, op1=ALU.add)
    t2 = fin.tile([128, MT], F32, tag="t2")
    nc.vector.tensor_scalar(out=t2[:], in0=u2[:], scalar1=11.0, scalar2=None,
                            op0=ALU.mult)
    nc.vector.tensor_tensor(out=t1[:], in0=t1[:], in1=t2[:], op=ALU.add)
    num = fin.tile([128, MT], F32, tag="num")
    nc.vector.tensor_tensor(out=num[:], in0=t1[:], in1=u[:], op=ALU.mult)
    den = fin.tile([128, MT], F32, tag="den")
    nc.vector.tensor_scalar(out=den[:], in0=u[:], scalar1=90.0, scalar2=60.0,
                            op0=ALU.mult, op1=ALU.add)
    nc.vector.tensor_scalar(out=t2[:], in0=u[:], scalar1=3.0, scalar2=36.0,
                            op0=ALU.mult, op1=ALU.add)
    nc.vector.tensor_tensor(out=t2[:], in0=t2[:], in1=u2[:], op=ALU.mult)
    nc.vector.tensor_tensor(out=den[:], in0=den[:], in1=t2[:], op=ALU.add)
    rden = fin.tile([128, MT], F32, tag="rden")
    nc.vector.reciprocal(rden[:], den[:])
    lg = fin.tile([128, MT], F32, tag="lg")
    nc.vector.tensor_tensor(out=lg[:], in0=num[:], in1=rden[:], op=ALU.mult)
    relu = fin.tile([128, MT], F32, tag="relu")
    nc.vector.tensor_scalar_max(out=relu[:], in0=xcol[:], scalar1=0.0)
    nc.vector.tensor_tensor(out=lg[:], in0=lg[:], in1=relu[:], op=ALU.add)
    lsum = fin.tile([128, 1], F32, tag="lsum")
    nc.vector.tensor_reduce(out=lsum[:], in_=lg[:],
                            axis=mybir.AxisListType.X, op=ALU.add)
    ps = mpsum.tile([128, 2048], F32, tag="grp", name="psfinal")
    nc.tensor.matmul(ps[0:1, 0:1], lsum[:], ones_col[:], start=True, stop=True)
    res = fin.tile([1, 1], F32, tag="res")
    nc.scalar.activation(res[:], ps[0:1, 0:1], AF.Copy)
    nc.sync.dma_start(out_d, res[:])


def _get_nc():
    if "nc" not in _CACHE:
        _CACHE["nc"] = _build()
    return _CACHE["nc"]


def _feedback_quant(x):
    """fp8-quantize rows of x with error feedback along the last axis so
    each row's fp8 sum tracks the fp32 row sum."""
    out = np.empty(x.shape, dtype=NPF8)
    carry = np.zeros(x.shape[0], dtype=np.float32)
    for d in range(x.shape[1]):
        v = x[:, d] + carry
        q = v.astype(NPF8)
        out[:, d] = q
        carry = v - q.astype(np.float32)
    return out


def _host_pack(A, P, N):
    Ys = [A, P, N]
    Y8 = [Y.astype(NPF8) for Y in Ys]
    Q8 = [_feedback_quant(Y * Y - 1.0) for Y in Ys]

    eye = (np.eye(128, dtype=np.float32) * -224.0).astype(NPF8)
    eyepack = np.ascontiguousarray(np.stack([eye, eye], axis=1))
    ib = np.zeros((128, 1024), dtype=np.float32)
    ib[:, 512:640] = np.eye(128, dtype=np.float32)
    ib8 = ib.astype(NPF8)
    ibufpack = np.ascontiguousarray(np.stack([ib8, ib8], axis=1))

    in_maps = []
    for c in range(NCORES):
        r = RB * c
        idx = np.r_[r:B, 0:r]
        m = {"eyepack": eyepack, "ibufpack": ibufpack}
        for y in range(3):
            v = Y8[y][idx].T
            q = Q8[y][idx].T
            m[f"ypk{y}"] = np.ascontiguousarray(np.stack([v, q], axis=1))
        ownT = Y8[0][idx][:RB].T          # [128, 512] fp8
        lhspack = np.empty((128, MT, 2, 128), dtype=NPF8)
        for mm in range(MT):
            lhspack[:, mm, 0, :] = ownT[:, 128 * mm:128 * (mm + 1)]
        lhspack[:, :, 1, :] = np.float32(-0.5)
        m["lhspack"] = lhspack
        m["aslice"] = np.ascontiguousarray(A[idx][:RB].T)
        m["pslice"] = np.ascontiguousarray(P[idx][:RB].T)
        in_maps.append(m)
    return in_maps


def kernel(rep_anchor, rep_pos, rep_neg):
    A = np.ascontiguousarray(rep_anchor, dtype=np.float32)
    P = np.ascontiguousarray(rep_pos, dtype=np.float32)
    N = np.ascontiguousarray(rep_neg, dtype=np.float32)

    nc = _get_nc()
    in_maps = _host_pack(A, P, N)
    res = bass_utils.run_bass_kernel_spmd(nc, in_maps,
                                          core_ids=list(range(NCORES)))
    total = np.float64(0.0)
    for c in range(NCORES):
        total += np.float64(res.results[c]["out"][0, 0])
    return np.float32(total / B)
